# revision 13
# baseline (speedup 1.0000x reference)
"""Dynamic conv2d (CondConv-style) Trainium2 Bass kernel, v2.

Problem: per-sample routing (GAP -> FC -> sigmoid over K=8 experts), expert
weight aggregation, then a per-sample 3x3 conv (pad=1) plus aggregated bias.

Sharding: data-parallel over batch across 8 NeuronCores (4 samples/core);
the K-expert weight bank is replicated to every core.

Per-core design (cost-model-driven):
  - 57-stride padded layout: row y's right pad IS row y+1's left pad (both
    zero), so chunks are 8 rows x 57 = 456 PSUM cols (1.75% waste vs 3.45%
    at 58).
  - x arrives in 4 row-chunks per sample; ONE ScalarE activation per chunk
    does cast (f32->f16) + padded re-layout + GAP partial sum (accum_out) in
    a single pass: no separate GAP pass, no DVE relay, no in-place staging
    write serializing the head.
  - Weight bank is stored tap-major per half ([ci, m, g, k, c]) so a
    3-tap "piece" of all 8 experts is one contiguous DMA; aggregation runs
    piece-wise (8 TSP products at the DVE's 4x mode + 7 fp16 adds at 2x per
    piece) and the conv starts as soon as piece 0 of m0 is aggregated.
  - Conv is piece-ordered: for each piece, all 7 row-chunks x 3 taps
    accumulate into 7 concurrently-open PSUM banks; extraction (ScalarE,
    fused bias add + interior 56-of-57 selection) fires per chunk after
    piece 2.
  - PE p-state warmup: the cost model charges a 2-4x slower PE clock for
    3us after any idle gap (sampled at dispatch). A chain of dummy matmuls
    keeps the ramp warm from t~0 until the first real conv matmul, and the
    routing matmuls r1..r3 are placed in the PE stream only where their
    deps are provably ready so the in-order engine never stalls.
"""

import numpy as np

B, C_IN, H, W = 32, 128, 56, 56
C_OUT, KS, K = 256, 3, 8
N_CORES = 8
B_LOC = B // N_CORES  # 4 samples per core

PW = W + 1                  # padded row stride: 57 (shared L/R pad zeros)
XBUF = (H + 2) * PW + 4     # 3310 -> pad to 3312
TAP_COLS = KS * KS * C_OUT  # 2304 aggregated-weight cols per sample
HW = H * W                  # 3136
M_TILES = C_OUT // 128      # 2
ROWS_PER_CHUNK = 8
N_ROW_CHUNKS = H // ROWS_PER_CHUNK  # 7
CW = ROWS_PER_CHUNK * PW            # 456 psum cols per chunk
OW = ROWS_PER_CHUNK * W             # 448 output cols per chunk
N_PIECES = 3                        # aggregation pieces (3 taps each)
PIECE_TAPS = KS * KS // N_PIECES    # 3
PIECE_COLS = PIECE_TAPS * K * 128   # 3072 bank cols per piece
AGG_PIECE = PIECE_TAPS * 128        # 384 aggregated cols per piece
XC = 4                              # x DMA chunks per sample
# smaller final chunk so the last relay (on the gap0 critical path) is short
XCHUNKS = [(0, 16), (16, 16), (32, 16), (48, 8)]  # (row0, nrows)

_CACHE = {}


def _make_tile_context_cls():
    import concourse.mybir as mybir
    from concourse.tile import TileContext
    from concourse.vector_clock import ScopedClock

    class SplitDrainTileContext(TileContext):
        """Walrus in this container caps sync waits per CTRL instruction;
        the Tile tail drain can accumulate more. Keep one wait on the drain
        and move the rest onto dedicated nops."""

        def _drain_and_barrier(self, tick_clock, wait_clock):
            drain_inst = self.nc.sync.drain()
            wait_clock.add_sem_waits(
                drain_inst.ins, ScopedClock({None: tick_clock.global_clock})
            )
            si = drain_inst.ins.sync_info
            if si is not None and len(si.on_wait) > 1:
                waits = list(si.on_wait)
                drain_inst.ins.sync_info = mybir.SyncInfo(
                    on_wait=waits[:1], on_update=list(si.on_update)
                )
                for w in waits[1:]:
                    n = self.nc.sync.nop(nofuse=True)
                    n.ins.sync_info = mybir.SyncInfo(on_wait=[w], on_update=[])
            self.nc.all_engine_barrier()
            assert self.sems is not None
            popped = self.nc._tile_sem_poison_stack.pop()
            assert popped is self._sem_poison
            self.nc.clear_and_free_semaphores(list(self.sems.allocated().values()))
            self.nc.all_engine_barrier()

    return SplitDrainTileContext


def _split_excess_waits(nc, cap=1):
    """The walrus build in this container rejects instructions carrying more
    than ~1-2 sem waits. Keep at most `cap` waits per instruction and move
    the rest onto same-engine NoOps inserted immediately before it."""
    import concourse.mybir as mybir

    for f in nc.m.functions:
        for blk in f.blocks:
            insts = blk.instructions
            if not any(
                i.sync_info is not None and len(i.sync_info.on_wait) > cap
                for i in insts
            ):
                continue
            new_insts = []
            for inst in insts:
                si = inst.sync_info
                if si is not None and len(si.on_wait) > cap:
                    waits = list(si.on_wait)
                    for j, w in enumerate(waits[cap:]):
                        noop = mybir.InstNoOp(
                            name=f"{inst.name}-waitsplit{j}",
                            engine=inst.engine,
                            ins=[],
                            outs=[],
                            bass_nofuse=True,
                            sync_info=mybir.SyncInfo(on_wait=[w], on_update=[]),
                        )
                        nc.register_instruction(noop)
                        new_insts.append(noop)
                    inst.sync_info = mybir.SyncInfo(
                        on_wait=waits[:cap], on_update=list(si.on_update)
                    )
                new_insts.append(inst)
            blk.instructions = new_insts


def _build_bass(reps=1, warm=(17, 30)):
    import concourse.bass as bass
    import concourse.mybir as mybir

    F32 = mybir.dt.float32
    F16 = mybir.dt.float16
    SIG = mybir.ActivationFunctionType.Sigmoid
    IDENT = mybir.ActivationFunctionType.Identity
    COPY = mybir.ActivationFunctionType.Copy
    MULT = mybir.AluOpType.mult
    ADD = mybir.AluOpType.add

    SplitDrainTileContext = _make_tile_context_cls()

    nc = bass.Bass()
    xs = nc.dram_tensor("xs", [B_LOC, C_IN, H, W], F32, kind="ExternalInput")
    wT = nc.dram_tensor("wT", [C_IN, M_TILES * K * TAP_COLS // 2], F16,
                        kind="ExternalInput")
    fcwT = nc.dram_tensor("fcwT", [C_IN, K], F32, kind="ExternalInput")
    fcb_bc = nc.dram_tensor("fcb_bc", [C_IN, K], F32, kind="ExternalInput")
    biasT = nc.dram_tensor("biasT", [C_OUT, K], F32, kind="ExternalInput")
    out = nc.dram_tensor("out", [B_LOC, C_OUT, H, W], F32, kind="ExternalOutput")

    inv_hw = 1.0 / float(HW)

    with SplitDrainTileContext(nc) as tc:
        with (
            tc.tile_pool(name="const", bufs=1) as constp,
            tc.tile_pool(name="xb", bufs=1) as xbp,
            tc.tile_pool(name="stg", bufs=8) as stgp,
            tc.tile_pool(name="agg", bufs=4) as aggp,
            tc.tile_pool(name="small", bufs=8) as smallp,
            tc.tile_pool(name="osb", bufs=2) as outp,
            tc.tile_pool(name="rps", bufs=1, space="PSUM") as rpsp,
            tc.tile_pool(name="cps", bufs=7, space="PSUM") as cpsp,
        ):
            # --- persistent tiles -------------------------------------
            # bank piece tiles: [m][p] of [128, 3072] fp16, tap-major
            # (g, k, c) so one piece of every expert is one contiguous DMA
            bank = [
                [
                    constp.tile([128, PIECE_COLS], F16,
                                name=f"bk{m}_{p}", tag=f"bk{m}_{p}")
                    for p in range(N_PIECES)
                ]
                for m in range(M_TILES)
            ]
            fcwT_sb = constp.tile([C_IN, K], F32, name="fcwT_sb", tag="fcwT")
            fcbbc_sb = constp.tile([C_IN, K], F32, name="fcbbc_sb", tag="fcbbc")
            biasT_sb = [
                constp.tile([128, K], F32, name=f"biasT{m}", tag=f"biasT{m}")
                for m in range(M_TILES)
            ]
            zeros128 = constp.tile([128, 128], F32, name="zeros128", tag="zeros")
            xbufs = [
                xbp.tile([128, XBUF], F16, name=f"xbuf{i}", tag=f"xbuf{i}")
                for i in range(B_LOC)
            ]
            tmps = [
                constp.tile([128, AGG_PIECE], F16, name=f"tmp{k}", tag=f"tmp{k}")
                for k in range(K)
            ]
            wdum = constp.tile([128, 464], F16, name="wdum", tag="wdum")
            wps = cpsp.tile([128, CW], F32, name="wps", tag="ps")

            # memset order = need order: wdum (warmup t~0.6), xbuf0
            # (relay0 t~4), zeros128 (gap_bc t~8.5), then the rest
            nc.gpsimd.memset(wdum[:, :], 0.0)
            nc.gpsimd.memset(xbufs[0][:, :], 0.0)
            nc.gpsimd.memset(zeros128[:, :], 0.0)
            for xb in xbufs[1:]:
                nc.gpsimd.memset(xb[:, :], 0.0)

            def warmup(n):
                for _ in range(n):
                    nc.tensor.matmul(
                        wps[:, 0:CW], lhsT=wdum[:, 0:128], rhs=wdum[:, 0:CW],
                        start=True, stop=True,
                    )

            # --- DMA issue helpers ------------------------------------
            stg_tiles = {}

            def load_x_chunk(b, c):
                row0, nrows = XCHUNKS[c]
                stg = stgp.tile([128, nrows * W], F32, name=f"stg{b}_{c}", tag="stg")
                nc.sync.dma_start(
                    out=stg[:, :],
                    in_=xs[b].rearrange("c h w -> c (h w)")[
                        :, row0 * W : (row0 + nrows) * W
                    ],
                )
                stg_tiles[(b, c)] = stg

            def load_bank(m, p):
                base = (m * N_PIECES + p) * PIECE_COLS
                nc.sync.dma_start(
                    out=bank[m][p][:, :], in_=wT[:, base : base + PIECE_COLS]
                )

            # --- per-sample stages ------------------------------------
            def relay_chunk(b, c, gpart):
                """ScalarE: cast f32->f16, write padded 57-stride layout,
                and accumulate the GAP partial sum, all in one pass."""
                stg = stg_tiles.pop((b, c))
                row0, nrows = XCHUNKS[c]
                start = (row0 + 1) * PW + 1
                dst = xbufs[b][:, start : start + nrows * PW].rearrange(
                    "p (y w) -> p y w", w=PW
                )[:, :, 0:W]
                nc.scalar.activation(
                    dst,
                    stg[:, :].rearrange("p (y w) -> p y w", w=W),
                    COPY, scale=1.0,
                    accum_out=gpart[:, c : c + 1],
                )

            def gap_combine(b, gpart):
                gap = smallp.tile([128, 1], F32, name=f"gap{b}", tag="gap")
                nc.vector.reduce_sum(
                    gap[:, 0:1], gpart[:, 0:XC], axis=mybir.AxisListType.X
                )
                return gap

            def gap_broadcast(b, gap):
                """DVE: broadcast gap along the free dim (emitted early so
                the PE routing matmul never waits on a late DVE slot)."""
                gap_bc = smallp.tile([128, 128], F32, name=f"gapbc{b}", tag="gapbc")
                nc.vector.tensor_scalar_add(gap_bc[:, :], zeros128[:, :], gap[:, 0:1])
                return gap_bc

            def route_mm(b, gap_bc):
                """PE: routing matmul, placed at a PE-stream slot where
                gap_bc is provably ready."""
                psB = rpsp.tile([128, K], F32, name=f"psB{b}", tag="rps")
                nc.tensor.matmul(
                    psB[:, 0:K], lhsT=gap_bc[:, 0:128], rhs=fcwT_sb[:, 0:K],
                    start=True, stop=True,
                )
                return psB

            def route_pre_stt(b, psB):
                pre = smallp.tile([128, K], F32, name=f"pre{b}", tag="pre")
                nc.vector.scalar_tensor_tensor(
                    out=pre[:, 0:K], in0=psB[:, 0:K], scalar=inv_hw,
                    in1=fcbbc_sb[:, 0:K], op0=MULT, op1=ADD,
                )
                return pre

            def route_sigmoid(b, pre):
                attn_bc = smallp.tile([128, K], F32, name=f"attnb{b}", tag="attnb")
                nc.scalar.activation(attn_bc[:, 0:K], pre[:, 0:K], SIG)
                return attn_bc

            def agg_bias(b, attn_bc):
                aggb = smallp.tile([128, M_TILES], F32, name=f"aggb{b}", tag="aggb")
                ttr = smallp.tile([128, K], F32, name=f"ttr{b}", tag="ttr")
                for m in range(M_TILES):
                    nc.vector.tensor_tensor(
                        out=ttr[:, 0:K], in0=biasT_sb[m][:, 0:K],
                        in1=attn_bc[:, 0:K], op=MULT,
                    )
                    nc.vector.reduce_sum(
                        aggb[:, m : m + 1], ttr[:, 0:K],
                        axis=mybir.AxisListType.X,
                    )
                return aggb

            def agg_piece(attn_bc, aggT, m, p):
                """DVE: 8 TSP products (4x mode) + 7 fp16 adds (2x mode)
                for one 3-tap piece of one output-channel half."""
                bview = bank[m][p][:, :].rearrange(
                    "q (g k c) -> q g k c", k=K, c=128
                )
                for k in range(K):
                    nc.vector.tensor_scalar_mul(
                        tmps[k][:, :].rearrange("q (g c) -> q g c", c=128),
                        bview[:, :, k, :],
                        attn_bc[:, k : k + 1],
                    )
                for k in range(0, K, 2):
                    nc.vector.tensor_tensor(
                        out=tmps[k][:, :], in0=tmps[k][:, :],
                        in1=tmps[k + 1][:, :], op=ADD,
                    )
                for k in range(0, K, 4):
                    nc.vector.tensor_tensor(
                        out=tmps[k][:, :], in0=tmps[k][:, :],
                        in1=tmps[k + 2][:, :], op=ADD,
                    )
                nc.vector.tensor_tensor(
                    out=aggT[:, p * AGG_PIECE : (p + 1) * AGG_PIECE],
                    in0=tmps[0][:, :], in1=tmps[4][:, :], op=ADD,
                )

            def conv_half_piece(b, m, p, aggT, ps_tiles):
                """PE: piece p's 3 taps for all 7 row chunks, accumulating
                into the 7 open PSUM banks."""
                for n in range(N_ROW_CHUNKS):
                    for j in range(PIECE_TAPS):
                        g = p * PIECE_TAPS + j
                        kh, kw = g // KS, g % KS
                        base = (ROWS_PER_CHUNK * n + kh) * PW + kw
                        nc.tensor.matmul(
                            ps_tiles[n][:, 0:CW],
                            lhsT=aggT[:, g * 128 : (g + 1) * 128],
                            rhs=xbufs[b][:, base : base + CW],
                            start=(p == 0 and j == 0),
                            stop=(p == N_PIECES - 1 and j == PIECE_TAPS - 1),
                        )

            def extract_half(b, m, osb, aggb, ps_tiles):
                for n in range(N_ROW_CHUNKS):
                    nc.scalar.activation(
                        osb[:, n * OW : (n + 1) * OW].rearrange(
                            "q (y w) -> q y w", w=W
                        ),
                        ps_tiles[n][:, 0:CW].rearrange(
                            "q (y w) -> q y w", w=PW
                        )[:, :, 0:W],
                        IDENT,
                        bias=aggb[:, m : m + 1], scale=1.0,
                    )
                    nc.sync.dma_start(
                        out=out[b, m * 128 : (m + 1) * 128].rearrange(
                            "c h w -> c (h w)"
                        )[:, n * OW : (n + 1) * OW],
                        in_=osb[:, n * OW : (n + 1) * OW],
                    )

            # --- schedule ---------------------------------------------
            aggTs, pss, osbs = {}, {}, {}

            def agg_pieces(b, m, attn, ps_list):
                if (b, m) not in aggTs:
                    aggTs[(b, m)] = aggp.tile(
                        [128, TAP_COLS // 2], F16, name=f"aggT{b}_{m}", tag="aggT"
                    )
                for p in ps_list:
                    agg_piece(attn, aggTs[(b, m)], m, p)

            def conv_pieces(b, m, ps_list):
                if (b, m) not in pss:
                    pss[(b, m)] = [
                        cpsp.tile([128, CW], F32, name=f"ps{b}_{m}_{n}", tag="ps")
                        for n in range(N_ROW_CHUNKS)
                    ]
                for p in ps_list:
                    conv_half_piece(b, m, p, aggTs[(b, m)], pss[(b, m)])

            def conv_chunks(b, m, chunks):
                """Chunk-major conv (all 9 taps per chunk): usable once the
                half's aggregation is fully done before the conv starts;
                lets each chunk's extraction+output DMA fire immediately so
                the final out-DMA burst doesn't serialize into the tail."""
                if (b, m) not in pss:
                    pss[(b, m)] = [
                        cpsp.tile([128, CW], F32, name=f"ps{b}_{m}_{n}", tag="ps")
                        for n in range(N_ROW_CHUNKS)
                    ]
                aggT = aggTs[(b, m)]
                for n in chunks:
                    for g in range(KS * KS):
                        kh, kw = g // KS, g % KS
                        base = (ROWS_PER_CHUNK * n + kh) * PW + kw
                        nc.tensor.matmul(
                            pss[(b, m)][n][:, 0:CW],
                            lhsT=aggT[:, g * 128 : (g + 1) * 128],
                            rhs=xbufs[b][:, base : base + CW],
                            start=(g == 0), stop=(g == KS * KS - 1),
                        )

            def extract(b, m, aggb):
                osbs[(b, m)] = outp.tile(
                    [128, HW], F32, name=f"osb{b}_{m}", tag="osb"
                )
                extract_half(b, m, osbs[(b, m)], aggb, pss[(b, m)])

            for _rep in range(reps):
                # DMA issue order (DMA_ENGINES is serialized, in-order):
                # x0 feeds the gap0 head; bank m0 pieces feed the first
                # aggregation; x1 and bank m1 interleave so gap1 and the
                # m1 aggregation both land just in time.
                for c in range(XC):
                    load_x_chunk(0, c)
                nc.sync.dma_start(out=fcwT_sb[:, :], in_=fcwT[:, :])
                nc.sync.dma_start(out=fcbbc_sb[:, :], in_=fcb_bc[:, :])
                for m in range(M_TILES):
                    nc.sync.dma_start(
                        out=biasT_sb[m][:, :], in_=biasT[m * 128 : (m + 1) * 128, :]
                    )
                load_bank(0, 0)
                load_bank(0, 1)
                load_x_chunk(1, 0)
                load_x_chunk(1, 1)
                load_bank(0, 2)
                load_x_chunk(1, 2)
                load_x_chunk(1, 3)
                load_bank(1, 0)
                load_bank(1, 1)
                load_bank(1, 2)
                for c in range(XC):
                    load_x_chunk(2, c)
                for c in range(XC):
                    load_x_chunk(3, c)

                gparts = [
                    smallp.tile([128, XC], F32, name=f"gp{b}", tag=f"gp{b % 2}")
                    for b in range(B_LOC)
                ]
                # s0 head: fused relay+GAP per chunk, then routing
                for c in range(XC):
                    relay_chunk(0, c, gparts[0])
                gap0 = gap_combine(0, gparts[0])
                gapbc0 = gap_broadcast(0, gap0)
                warmup(warm[0])
                psB0 = route_mm(0, gapbc0)
                pre0 = route_pre_stt(0, psB0)
                attn0 = route_sigmoid(0, pre0)

                # DVE: s0 m0 aggregation piece-wise
                agg_pieces(0, 0, attn0, [0, 1, 2])
                aggb0 = agg_bias(0, attn0)
                # ACT: x1 relays before s0 extractions
                for c in range(XC):
                    relay_chunk(1, c, gparts[1])
                gap1 = gap_combine(1, gparts[1])
                agg_pieces(0, 1, attn0, [0])
                gapbc1 = gap_broadcast(1, gap1)

                warmup(warm[1])
                conv_pieces(0, 0, [0, 1, 2])
                extract(0, 0, aggb0)
                psB1 = route_mm(1, gapbc1)
                pre1 = route_pre_stt(1, psB1)
                agg_pieces(0, 1, attn0, [1, 2])
                attn1 = route_sigmoid(1, pre1)
                conv_pieces(0, 1, [0, 1, 2])
                agg_pieces(1, 0, attn1, [0, 1, 2])
                aggb1 = agg_bias(1, attn1)
                extract(0, 1, aggb0)
                # ACT: x2 relays; DVE: gap2 prep
                for c in range(XC):
                    relay_chunk(2, c, gparts[2])
                gap2 = gap_combine(2, gparts[2])
                gapbc2 = gap_broadcast(2, gap2)

                conv_chunks(1, 0, range(0, 4))
                psB2 = route_mm(2, gapbc2)
                pre2 = route_pre_stt(2, psB2)
                conv_chunks(1, 0, range(4, N_ROW_CHUNKS))
                extract(1, 0, aggb1)
                attn2 = route_sigmoid(2, pre2)
                agg_pieces(1, 1, attn1, [0, 1, 2])
                conv_chunks(1, 1, range(N_ROW_CHUNKS))
                agg_pieces(2, 0, attn2, [0, 1, 2])
                aggb2 = agg_bias(2, attn2)
                extract(1, 1, aggb1)
                for c in range(XC):
                    relay_chunk(3, c, gparts[3])
                gap3 = gap_combine(3, gparts[3])
                gapbc3 = gap_broadcast(3, gap3)

                conv_chunks(2, 0, range(0, 4))
                psB3 = route_mm(3, gapbc3)
                pre3 = route_pre_stt(3, psB3)
                conv_chunks(2, 0, range(4, N_ROW_CHUNKS))
                extract(2, 0, aggb2)
                attn3 = route_sigmoid(3, pre3)
                agg_pieces(2, 1, attn2, [0, 1, 2])
                conv_chunks(2, 1, range(N_ROW_CHUNKS))
                agg_pieces(3, 0, attn3, [0, 1, 2])
                aggb3 = agg_bias(3, attn3)
                extract(2, 1, aggb2)

                conv_chunks(3, 0, range(N_ROW_CHUNKS))
                extract(3, 0, aggb3)
                agg_pieces(3, 1, attn3, [0, 1, 2])
                conv_chunks(3, 1, range(N_ROW_CHUNKS))
                extract(3, 1, aggb3)

    _split_excess_waits(nc)
    return nc


def _get_nc():
    if "nc" not in _CACHE:
        _CACHE["nc"] = _build_bass()
    return _CACHE["nc"]


def _host_prep(fc_w, fc_b, weight, bias):
    # bank layout: [ci, m, g(kh*3+kw), k, c128] fp16, so each (m, piece)
    # is one contiguous [128, 3072] DMA and expert k within a piece is a
    # clean strided view
    w6 = weight.astype(np.float32).reshape(K, M_TILES, 128, C_IN, KS, KS)
    wT_host = np.ascontiguousarray(
        w6.transpose(3, 1, 4, 5, 0, 2)
    ).reshape(C_IN, M_TILES * K * TAP_COLS // 2).astype(np.float16)
    return {
        "wT": wT_host,
        "fcwT": np.ascontiguousarray(fc_w.astype(np.float32).T),
        "fcb_bc": np.ascontiguousarray(
            np.tile(fc_b.astype(np.float32).reshape(1, K), (C_IN, 1))
        ),
        "biasT": np.ascontiguousarray(bias.astype(np.float32).T),
    }


def kernel(x, fc_w, fc_b, weight, bias):
    from concourse.bass_utils import run_bass_kernel_spmd

    x = np.asarray(x)
    fc_w, fc_b = np.asarray(fc_w), np.asarray(fc_b)
    weight, bias = np.asarray(weight), np.asarray(bias)

    nc = _get_nc()
    shared = _host_prep(fc_w, fc_b, weight, bias)
    x = np.ascontiguousarray(x.astype(np.float32))
    in_maps = [
        {"xs": x[c * B_LOC : (c + 1) * B_LOC], **shared} for c in range(N_CORES)
    ]
    res = run_bass_kernel_spmd(nc, in_maps, core_ids=list(range(N_CORES)))
    _CACHE["last_res"] = res
    return np.concatenate([r["out"] for r in res.results], axis=0)


if __name__ == "__main__":
    rng = np.random.default_rng(0)
    x = rng.standard_normal((B, C_IN, H, W), dtype=np.float32)
    fc_w = rng.standard_normal((K, C_IN), dtype=np.float32) * 0.05
    fc_b = rng.standard_normal((K,), dtype=np.float32) * 0.05
    weight = rng.standard_normal((K, C_OUT, C_IN, KS, KS), dtype=np.float32) * 0.05
    bias = rng.standard_normal((K, C_OUT), dtype=np.float32) * 0.05
    out = kernel(x, fc_w, fc_b, weight, bias)
    print(out.shape, out.dtype, np.abs(out).mean())
